# revision 1
# baseline (speedup 1.0000x reference)
"""Cross-activation regularization loss kernel for Trainium2 (8 NeuronCores).

Data-parallel over the batch: each core loads its 8 samples of
x[B=64, T=4096, F=128] once, as fp8e4 (SWDGE cast in the DMA datapath, so
the modeled DMA write traffic is 1 byte/elem), and computes per sample in
one pass (the only O(B*T*F) work in the problem):
    G  = x_b^T x_b          Gram over time  [F,F]   (PE DoubleRow matmuls)
    s' = x_b^T (1/sqrt(T))  scaled column sums      (PE, same stationary)
    l1 = |x_b|^T (1/sqrt(T))  via DVE sign-bit mask + PE column sums
The per-sample [G | s' | l1] blocks stream back to the host as f32, which
finishes the O(B*F^2) correlation normalization in float64:
    d = diag(G) - s'^2;  r = d^-1/2;  v = s' * r
    corr_sum += G * r r^T - v v^T  ->  tri_sum, loss;  l1 -> activity_l1.

fp8e4 input quantization keeps the end-to-end error ~7e-4 (26x inside the
2e-2 gate) on both CPU- and device-generated reference inputs.

Scheduling notes (engine queues are in-order; the Tile list scheduler
follows emission priority):
  - per sample only [DMA] -> [matmuls] -> [copy] -> [output DMA]; no
    cross-engine scaling chains, so nothing queues behind unmet deps;
  - concurrently-open matmul accumulation groups must live in different
    PSUM banks: interleaving two groups in one bank corrupts the
    accumulation on real HW (verified by direct probe);
  - SWDGE descriptor generation (~1us/DMA) must stay under the per-DMA
    transfer time or the Pool engine paces the stream (so split=1);
  - the last sample's load is split so the post-stream tail only waits on
    half a sample of Gram work.
"""

from contextlib import ExitStack

import numpy as np

import concourse.bacc as bacc
import concourse.bass_utils as bass_utils
import concourse.tile as tile
from concourse import mybir
from concourse.bass import ts

N_CORES = 8
B = 64
B_PER_CORE = B // N_CORES  # 8
T = 4096
F = 128
NCHUNK = T // 128  # 32
CROSS_ACTIVATION_LAMBDA = 0.01

_f32 = mybir.dt.float32
_bf16 = mybir.dt.bfloat16
_fp8 = mybir.dt.float8e4
_u16 = mybir.dt.uint16

GCOLS = F + 2  # per-sample output block: [G | s' | l1col]
LASTSPLIT = [2]
LASTCH = [16]


def _build(loop_m=None, dtype="bf16", split=2, xbufs=4, pre=2, drow=False):
    nc = bacc.Bacc("TRN2", target_bir_lowering=False, debug=False,
                   num_devices=N_CORES)
    x_d = nc.dram_tensor("x_local", [B_PER_CORE, T, F], _f32,
                         kind="ExternalInput")
    # sample PAIRS per output block: 2*GCOLS bf16 = 520 B/partition row
    # keeps the DMA descriptor run >= 512 B at half the f32 traffic
    g_d = nc.dram_tensor("gout", [B_PER_CORE // 2, F, 2 * GCOLS], _bf16,
                         kind="ExternalOutput")

    fp8 = dtype == "fp8"
    xdt = _fp8 if fp8 else _bf16
    # t = 32*p + n: partition p holds 32 consecutive time rows (contiguous
    # DMA); chunk n contracts t over partitions, summing chunks covers all t.
    xr = x_d.ap().rearrange("b (p n) f -> b p (n f)", p=128)

    with tile.TileContext(nc) as tc, ExitStack() as ctx:
        singles = ctx.enter_context(tc.tile_pool(name="singles", bufs=1))
        xpool = ctx.enter_context(tc.tile_pool(name="x", bufs=xbufs))
        apool = ctx.enter_context(tc.tile_pool(name="a", bufs=2))
        gout = ctx.enter_context(tc.tile_pool(name="gout", bufs=8))
        psG = ctx.enter_context(tc.tile_pool(name="psG", bufs=3, space="PSUM"))
        psS = ctx.enter_context(tc.tile_pool(name="psS", bufs=1, space="PSUM"))
        psL = ctx.enter_context(tc.tile_pool(name="psL", bufs=1, space="PSUM"))

        cv = singles.tile([128, 2 if drow else 1], xdt)
        nc.vector.memset(cv, 1.0 / 64.0)  # 1/sqrt(T), exactly representable
        # head trick: the first SWDGE transfer can't start before ~2.4us
        # (preamble + descriptor-gen + DGE); an HWDGE fp32 load of the
        # first 2 chunks fills that idle DMA window and shortens the fp8
        # stream by the same bytes. Those chunks run as plain fp32 matmuls.
        h32 = fp8 and drow
        if h32:
            xf0 = singles.tile([128, 2 * F], _f32)
            af0 = singles.tile([128, 2 * F], _f32)
            cv32 = singles.tile([128, 1], _f32)
            nc.vector.memset(cv32, 1.0 / 64.0)
        # NB: concurrently-open matmul accumulation groups must live in
        # DIFFERENT PSUM banks -- interleaving two groups in one bank
        # corrupts the accumulation on real HW (verified by probe).
        sp_ps = psS.tile([F, B_PER_CORE], _f32)  # s' column per sample
        lc_ps = psL.tile([F, B_PER_CORE], _f32)  # |x| column per sample
        # finer pieces for the tail samples tighten the post-stream chain
        splits = [split] * (B_PER_CORE - 1) + [max(split, LASTSPLIT[0])]

        def pieces(b):
            if b == B_PER_CORE - 1 and splits[b] == 2:
                # uneven split: a small 4-chunk final piece (512 B/partition
                # descriptor runs, still mult-1) minimizes the post-stream
                # Gram work on the critical tail
                return [slice(0, (NCHUNK - LASTCH[0]) * F),
                        slice((NCHUNK - LASTCH[0]) * F, NCHUNK * F)]
            s = splits[b]
            w = NCHUNK // s * F
            return [slice(k * w, min((k + 1) * w, NCHUNK * F))
                    for k in range((NCHUNK * F + w - 1) // w)]

        def issue_dma(b, xb):
            for sl in pieces(b):
                # SWDGE casts fp32->bf16/fp8 in the DMA datapath
                nc.gpsimd.dma_start(out=xb[:, sl], in_=xr[b][:, sl])

        loop_cm = tc.For_i(0, loop_m, 1) if loop_m is not None else None
        if loop_cm is not None:
            loop_cm.__enter__()

        xbs = {}
        if h32:
            nc.sync.dma_start(out=xf0, in_=xr[0][:, :2 * F])
        for b in range(pre):
            xbs[b] = xpool.tile([128, NCHUNK * F], xdt, tag="xb",
                                name=f"xb{b}")
            if h32 and b == 0:
                for sl in pieces(0):
                    s0 = max(sl.start, 2 * F)
                    if s0 < sl.stop:
                        nc.gpsimd.dma_start(out=xbs[b][:, s0:sl.stop],
                                            in_=xr[0][:, s0:sl.stop])
            else:
                issue_dma(b, xbs[b])

        for b in range(B_PER_CORE):
            if b + pre < B_PER_CORE:
                xbs[b + pre] = xpool.tile([128, NCHUNK * F], xdt, tag="xb",
                                          name=f"xb{b + pre}")
                issue_dma(b + pre, xbs[b + pre])
            xb = xbs.pop(b)
            gs = psG.tile([F, F], _f32, tag="G")   # Gram accumulator
            sp = sp_ps[:, b:b + 1]
            lc = lc_ps[:, b:b + 1]
            # |x| via sign-bit clear on a uint16 view (one bf16 or two
            # fp8 lanes per element) at the 4x DVE rate; summed on the PE
            # into the lc PSUM column alongside the Gram matmuls.
            lanes = 2 if fp8 else 1
            mask = 0x7F7F if fp8 else 0x7FFF
            ab = apool.tile([128, NCHUNK * F // lanes], _u16, tag="ab")
            abx = ab.bitcast(xdt)
            for sl in pieces(b):
                nice = (sl.stop - sl.start) // F
                k0 = sl.start // F
                a0 = sl.start + (2 * F if (h32 and b == 0 and k0 == 0)
                                 else 0)  # fp32 head chunks not in xb
                slh = slice(a0 // lanes, sl.stop // lanes)
                nc.vector.tensor_scalar(
                    out=ab[:, slh], in0=xb.bitcast(_u16)[:, slh],
                    scalar1=mask, scalar2=None,
                    op0=mybir.AluOpType.bitwise_and)
                if drow:
                    # fp8 DoubleRow: two 128-row k-tiles per instruction.
                    # lc matmuls go after G/sp so PE never queues behind the
                    # DVE |x| pass.
                    x3 = xb.rearrange("p (n f) -> p n f", f=F)
                    a3 = abx.rearrange("p (n f) -> p n f", f=F)
                    c3 = cv.rearrange("p (n f) -> p n f", f=1)
                    dm = mybir.MatmulPerfMode.DoubleRow
                    fpp = 2 if (h32 and b == 0 and k0 == 0) else 0
                    if fpp:
                        # first 2 chunks arrived fp32 via the HWDGE head DMA
                        nc.vector.tensor_scalar(
                            out=af0.bitcast(mybir.dt.uint32),
                            in0=xf0.bitcast(mybir.dt.uint32),
                            scalar1=0x7FFFFFFF, scalar2=None,
                            op0=mybir.AluOpType.bitwise_and)
                        for n in (0, 1):
                            nc.tensor.matmul(gs, xf0[:, ts(n, F)],
                                             xf0[:, ts(n, F)],
                                             start=(n == 0), stop=False,
                                             skip_group_check=True)
                            nc.tensor.matmul(sp, xf0[:, ts(n, F)], cv32,
                                             start=(n == 0), stop=False,
                                             skip_group_check=True)
                    for n in range(k0 + fpp, k0 + nice, 2):
                        st, en = (n == 0), (n == NCHUNK - 2)
                        pair = x3[:, n:n + 2, :]
                        nc.tensor.matmul(gs, pair, pair,
                                         start=st, stop=en, perf_mode=dm,
                                         skip_group_check=True)
                        nc.tensor.matmul(sp, pair, c3,
                                         start=st, stop=en, perf_mode=dm,
                                         skip_group_check=True)
                    if fpp:
                        for n in (0, 1):
                            nc.tensor.matmul(lc, af0[:, ts(n, F)], cv32,
                                             start=(n == 0), stop=False,
                                             skip_group_check=True)
                    for n in range(k0 + fpp, k0 + nice, 2):
                        nc.tensor.matmul(lc, a3[:, n:n + 2, :], c3,
                                         start=(n == 0), stop=(n == NCHUNK - 2),
                                         perf_mode=dm, skip_group_check=True)
                else:
                    for n in range(k0, k0 + nice):
                        chunk = xb[:, ts(n, F)]
                        nc.tensor.matmul(gs, chunk, chunk,
                                         start=(n == 0),
                                         stop=(n == NCHUNK - 1))
                        nc.tensor.matmul(sp, chunk, cv,
                                         start=(n == 0),
                                         stop=(n == NCHUNK - 1))
                    for n in range(k0, k0 + nice):
                        nc.tensor.matmul(lc, abx[:, ts(n, F)], cv,
                                         start=(n == 0),
                                         stop=(n == NCHUNK - 1))
            if b % 2 == 0:
                g_sb = gout.tile([F, 2 * GCOLS], _bf16, tag="g_sb",
                                 name=f"g_sb{b}")
            off = (b % 2) * GCOLS
            nc.scalar.copy(out=g_sb[:, off:off + F], in_=gs)  # ACT, PSUM->SBUF
            nc.vector.tensor_copy(out=g_sb[:, off + F:off + F + 1], in_=sp)
            nc.vector.tensor_copy(out=g_sb[:, off + F + 1:off + F + 2], in_=lc)
            if b % 2 == 1:
                nc.sync.dma_start(out=g_d.ap()[b // 2], in_=g_sb)

        if loop_cm is not None:
            loop_cm.__exit__(None, None, None)

    nc.compile()
    return nc


_nc_cache = None
_DTYPE = "fp8"
_CFG = dict(dtype=_DTYPE, drow=_DTYPE == "fp8",
            split=1 if _DTYPE == "fp8" else 2, xbufs=6)


def _get_nc():
    global _nc_cache
    if _nc_cache is None:
        _nc_cache = _build(**_CFG)
    return _nc_cache


def _run(x, **spmd_kwargs):
    x = np.ascontiguousarray(np.asarray(x, dtype=np.float32))
    assert x.shape == (B, T, F), x.shape
    nc = _get_nc()
    in_maps = [{"x_local": x[c * B_PER_CORE:(c + 1) * B_PER_CORE]}
               for c in range(N_CORES)]
    return bass_utils.run_bass_kernel_spmd(
        nc, in_maps, core_ids=list(range(N_CORES)), **spmd_kwargs)


def _finalize(results):
    corr = np.zeros((F, F), dtype=np.float64)
    l1 = 0.0
    for res in results:
        go = res["gout"].astype(np.float64)          # [B/2, F, 2*GCOLS] pairs
        go = go.reshape(B_PER_CORE // 2, F, 2, GCOLS).transpose(0, 2, 1, 3)
        go = go.reshape(B_PER_CORE, F, GCOLS)
        G = go[:, :, :F]                             # sum_t x x^T
        s = go[:, :, F]                              # sum_t x / 64
        d = np.einsum('bff->bf', G) - s * s          # T * var
        r = 1.0 / np.sqrt(d)
        corr += np.einsum('bfg,bf,bg->fg', G, r, r)
        v = s * r
        corr -= np.einsum('bf,bg->fg', v, v)         # mean-centering term
        l1 += 64.0 * float(go[:, :, F + 1].sum())
    avg_abs = np.abs(corr / B)
    tri_sum = float(np.triu(avg_abs, k=1).sum())
    n_pairs = F * (F - 1) // 2
    loss = tri_sum * CROSS_ACTIVATION_LAMBDA / n_pairs
    activity_l1 = l1 / F
    return np.array([loss, tri_sum, activity_l1], dtype=np.float32)


def kernel(inputs):
    br = _run(inputs)
    return _finalize(br.results)

